# revision 1
# baseline (speedup 1.0000x reference)
"""Trainium2 Bass kernel for nn_Conv1d_NN (retrieval_knn).

Math (per batch element b of x [B=4, C=16, N=8192]):
  dist[n,m] = |x[:,n] - x[:,m]|^2, diag forced to be the nearest neighbor,
  idx[n,:]  = indices of the 9 smallest distances (sorted, ties -> lower idx),
  out[o,n]  = sum_{k,c} W[o,c,k] * x[c, idx[n,k]] + bias[o].

Sharding: 8 cores = 4 batches x 2 row-halves (data parallel over batch,
sequence parallel over distance-matrix rows). Each core receives its batch's
x ROLLED so its rows are always local rows 0..4095 — this keeps the SPMD
program identical on every core (static diag positions). Indices stay in
rolled coordinates, so the neighbor gather is roll-consistent and the output
unrolls trivially on the host.

Device algorithm per core (32 row-tiles of 128 rows):
  1. PE computes  neg[n,m] = 2*dot - ns[n] - ns[m]  (= -dist) directly via an
     augmented 18-row contraction (rows: 2x / ones / ns  against  x / -ns / -ones).
  2. ScalarE drains PSUM -> SBUF; one [128,128] add kills the diagonal (-1e30).
  3. VectorE max/max_index produce the top-8 values+indices per row exactly
     (first-unused-occurrence matching == jax.lax.top_k tie-breaking).
     Neighbor k=0 is always the row itself (diag) -> no compute needed.
  4. Indices are stream-transposed and DMA-rearranged into the 16-partition
     wrapped layout of gpsimd ap_gather; each of the 8 Q7 cores gathers the
     columns for its own k in {1..8} from an 8x-replicated copy of x.
  5. One K=128 matmul contracts all 8 gathered neighbors against the
     flattened conv weights; a second K=18 matmul adds the k=0 term and the
     bias. ScalarE un-permutes the PSUM into the output buffer.
"""
import sys

if "/opt/trn_rl_repo" not in sys.path:
    sys.path.insert(0, "/opt/trn_rl_repo")

import numpy as np

_N = 8192
_C = 16
_B = 4
_NCORES = 8
_HALF = _N // 2          # rows per core
_TILE = 128
_T = _HALF // _TILE      # 32 row-tiles per core
_CHUNK = 512
_NCHUNK = _N // _CHUNK   # 16 Gram chunks per row-tile

_prog = None


def _build():
    global _prog
    if _prog is not None:
        return _prog
    from contextlib import ExitStack

    import concourse.bacc as bacc
    import concourse.mybir as mybir
    from concourse import tile

    dt = mybir.dt
    nc = bacc.Bacc("TRN2", target_bir_lowering=False, debug=False,
                   num_devices=_NCORES)

    xa_d = nc.dram_tensor("xa", [18, _N], dt.float32, kind="ExternalInput")
    xar_d = nc.dram_tensor("xar", [18, _N], dt.float32, kind="ExternalInput")
    wf_d = nc.dram_tensor("wf", [128, 16], dt.float32, kind="ExternalInput")
    w0b_d = nc.dram_tensor("w0b", [18, 16], dt.float32, kind="ExternalInput")
    dk_d = nc.dram_tensor("dk", [128, _TILE], dt.float32, kind="ExternalInput")
    out_d = nc.dram_tensor("out", [16, _HALF], dt.float32, kind="ExternalOutput")

    with tile.TileContext(nc) as tc, ExitStack() as ctx:
        constp = ctx.enter_context(tc.tile_pool(name="const", bufs=1))
        negp = ctx.enter_context(tc.tile_pool(name="negd", bufs=2))
        psg = ctx.enter_context(tc.tile_pool(name="psg", bufs=4, space="PSUM"))
        psc = ctx.enter_context(tc.tile_pool(name="psc", bufs=2, space="PSUM"))
        small = ctx.enter_context(tc.tile_pool(name="small", bufs=3))
        outp = ctx.enter_context(tc.tile_pool(name="outp", bufs=1))

        XA = constp.tile([18, _N], dt.float32)    # [2x; ones; ns]
        XAR = constp.tile([18, _N], dt.float32)   # [x; -ns; -ones]
        X8 = constp.tile([128, _N], dt.float32)   # x replicated on 8 groups
        WF = constp.tile([128, 16], dt.float32)   # W[o,c,k] for k=1..8, flat
        W0B = constp.tile([18, 16], dt.float32)   # [W0^T/2; bias; 0]
        DK = constp.tile([128, _TILE], dt.float32)  # -1e30 on the diagonal
        OUT = outp.tile([16, _HALF], dt.float32)

        nc.sync.dma_start(out=XA[:], in_=xa_d.ap())
        nc.sync.dma_start(out=XAR[:], in_=xar_d.ap())
        nc.sync.dma_start(out=WF[:], in_=wf_d.ap())
        nc.sync.dma_start(out=W0B[:], in_=w0b_d.ap())
        nc.sync.dma_start(out=DK[:], in_=dk_d.ap())
        for g in range(8):
            nc.sync.dma_start(out=X8[16 * g:16 * (g + 1), :],
                              in_=xar_d.ap()[0:16, :])

        for t in range(_T):
            r0 = t * _TILE
            negd = negp.tile([128, _N], dt.float32)
            # --- negative squared distances for 128 rows x all 8192 cols ---
            for cc in range(_NCHUNK):
                ps = psg.tile([128, _CHUNK], dt.float32, tag="psg")
                nc.tensor.matmul(ps[:], XA[:, r0:r0 + _TILE],
                                 XAR[:, _CHUNK * cc:_CHUNK * (cc + 1)],
                                 start=True, stop=True)
                nc.scalar.copy(negd[:, _CHUNK * cc:_CHUNK * (cc + 1)], ps[:])
            # kill the diagonal so max8 yields the 8 nearest non-self columns
            nc.vector.tensor_add(negd[:, r0:r0 + _TILE],
                                 negd[:, r0:r0 + _TILE], DK[:])
            # --- exact top-8 (values then indices) per row ---
            vals8 = small.tile([128, 8], dt.float32, tag="vals8")
            nc.vector.max(vals8[:], negd[:])
            idxp = small.tile([128, 32], dt.uint32, tag="idxp")
            nc.vector.memset(idxp[:, 8:32], 0)
            nc.vector.max_index(idxp[:, 0:8], vals8[:], negd[:])
            # --- rearrange indices into ap_gather's wrapped layout ---
            idxpT = small.tile([128, 32], dt.uint32, tag="idxpT")
            nc.vector.transpose(idxpT[:], idxp[:])
            wrapped32 = small.tile([128, 8], dt.uint32, tag="wrapped32")
            for g in range(8):
                for ph in range(4):
                    src = idxpT[32 * ph + g:32 * ph + g + 1, :] \
                        .rearrange("q (pl s) -> q pl s", pl=4)
                    dst = wrapped32[16 * g + 4 * ph:16 * g + 4 * ph + 4, 0:8]
                    nc.sync.dma_start(out=dst, in_=src)
            wrapped = small.tile([128, 8], dt.int16, tag="wrapped")
            nc.vector.tensor_copy(wrapped[:], wrapped32[:])
            # --- gather neighbors k=1..8: group g fetches x[:, idx[n, k=g+1]] ---
            gath = small.tile([128, _TILE], dt.float32, tag="gath")
            nc.gpsimd.ap_gather(gath[:], X8[:], wrapped[:], channels=128,
                                num_elems=_N, d=1, num_idxs=_TILE)
            # --- conv: one K=128 matmul over (k=1..8, c) + K=18 for k=0+bias ---
            pc = psc.tile([16, _TILE], dt.float32, tag="psc")
            nc.tensor.matmul(pc[:], WF[:], gath[:], start=True, stop=False)
            rhs0 = XA[:, r0:r0 + _TILE].rearrange("c (p s) -> c s p", s=8)
            nc.tensor.matmul(pc[:], W0B[:], rhs0, start=False, stop=True)
            # un-permute the pi(i)=8*(i%16)+i//16 column order while draining
            outap = OUT[:, r0:r0 + _TILE].rearrange("o (p s) -> o s p", s=8)
            nc.scalar.copy(outap, pc[:].rearrange("o (s p) -> o s p", p=16))

        nc.sync.dma_start(out=out_d.ap(), in_=OUT[:])

    nc.compile()
    _prog = nc
    return nc


def _host_prep(x, W, b):
    x = np.asarray(x, dtype=np.float32)
    W = np.asarray(W, dtype=np.float32)
    b = np.asarray(b, dtype=np.float32)

    wf = np.zeros((128, 16), np.float32)
    for g in range(8):
        wf[16 * g:16 * (g + 1), :] = W[:, :, g + 1].T
    w0b = np.zeros((18, 16), np.float32)
    w0b[0:16, :] = W[:, :, 0].T / 2.0
    w0b[16, :] = b
    dk = np.zeros((128, _TILE), np.float32)
    np.fill_diagonal(dk, np.float32(-1e30))

    in_maps = []
    for core in range(_NCORES):
        bi, h = divmod(core, 2)
        xr = np.roll(x[bi], -h * _HALF, axis=1)
        ns = (xr.astype(np.float64) ** 2).sum(0).astype(np.float32)
        xa = np.empty((18, _N), np.float32)
        xa[0:16] = 2.0 * xr
        xa[16] = 1.0
        xa[17] = ns
        xar = np.empty((18, _N), np.float32)
        xar[0:16] = xr
        xar[16] = -ns
        xar[17] = -1.0
        in_maps.append({"xa": xa, "xar": xar, "wf": wf, "w0b": w0b, "dk": dk})
    return in_maps


def kernel(x, W, b):
    nc = _build()
    from concourse.bass_utils import run_bass_kernel_spmd

    in_maps = _host_prep(x, W, b)
    res = run_bass_kernel_spmd(nc, in_maps, list(range(_NCORES)))
    out = np.zeros((_B, _C, _N), np.float32)
    for core in range(_NCORES):
        bi, h = divmod(core, 2)
        out[bi, :, h * _HALF:(h + 1) * _HALF] = res.results[core]["out"]
    return out
